# revision 49
# baseline (speedup 1.0000x reference)
"""Trainium2 Bass kernel for the Householder-chain problem.

Computes y = x @ Q.T where Q = M_0 @ M_1 @ ... @ M_{N-1} is a product of
N=514 Householder reflections M_i = I - 2 v_i v_i^T / (v_i^T v_i + eps)
over S=512 dims, and x is [65536, 512].

Math: since each M_i is symmetric, Q.T = M_{N-1} @ ... @ M_0 =: A, and the
product collapses via the compact-WY representation with natural column
order:  A = I - V T V^T  where V = [v_0 ... v_{N-1}] (S x N) and
T^{-1} = R = stril(V^T V) + diag((||v_i||^2 + eps)/2)   (lower triangular).

A is tiny (512 x 512) and depends only on `vectors`, so it is computed
once on the host in float64 (exact to ~1e-15; the end-to-end error budget
is set by bf16 rounding below, ~3e-3 against a 2e-2 gate).

Device work is the single streaming matmul y = x @ A, data-parallel over
the 65536 rows across 8 cores (8192 rows/core), all in bf16 with fp32
PSUM accumulation (bf16 halves HBM traffic vs fp32 and runs the PE at
1 column/cycle with hidden fast-weight-loads).

Layouts are packed on the host so every DMA is one large contiguous
transfer: x arrives as per-chunk [128, 4*w] blocks (x^T tiles for the 4
contraction sub-blocks side by side), y leaves in groups of 8 row-tiles
as [128, 8*512] blocks (~1 MiB per DMA).  The first chunks are small so
the PE starts within a few microseconds, and a handful of throwaway
matmuls at t=0 trip the PE clock-gate (HAM) to full rate while the first
DMAs are still in flight.
"""

from contextlib import ExitStack

import ml_dtypes
import numpy as np

import bass_rust
import concourse.bass as bass
import concourse.mybir as mybir
import concourse.tile as tile
from concourse.bass_utils import run_bass_kernel_spmd
from concourse.vector_clock import ScopedClock

FP = mybir.dt.float32
BF = mybir.dt.bfloat16

S = 512           # feature dim
NV = 514          # number of householder vectors
B = 65536         # batch rows
NCORES = 8
BPC = B // NCORES  # 8192 rows per core
EPS = 1e-16
BF_NP = ml_dtypes.bfloat16

CHUNKS = [128, 128, 256, 512, 1024, 2048, 2048, 2048]  # batch cols per chunk
W0 = CHUNKS[0]    # first chunk rides in the packed first-block DMA with A
assert sum(CHUNKS) == BPC
LOOKAHEAD = 4     # chunk c's body emits the DMA for chunk c+LOOKAHEAD
# y row-tiles per output DMA; smaller final groups shorten the tail store
# ([4,2,1,1] with a split last copy measured ~1 us slower - more store
# issues outweigh the shorter final transfer)
YGRPS = [8] * 7 + [4, 2, 2]
assert sum(YGRPS) * 128 == BPC
WARM_MM = 13      # PE prewarm matmuls (256 cols each, ~3.4 us cold = the
                  # HAM busy-window length) during the initial DMA wait;
                  # 11 (ending at the split block's first half) measured
                  # no better - the shorter warm leaves the HAM window
                  # unfilled and the early real matmuls run cold


# ---------------------------------------------------------------------------
# walrus CTRL instructions accept at most 4 sem waits, and this Tile
# version puts the whole global-clock wait set on the single tail drain.
# Spread the waits over preceding SP nops (1 wait each, conservatively).
def _patched_drain_and_barrier(self, tick_clock, wait_clock):
    pre_nops = [self.nc.sync.nop() for _ in range(12)]
    drain_inst = self.nc.sync.drain()
    wait_clock.add_sem_waits(
        drain_inst.ins, ScopedClock({None: tick_clock.global_clock})
    )
    si = drain_inst.ins.sync_info
    waits = list(si.on_wait) if si is not None and si.on_wait else []
    if len(waits) > 1:
        assert len(waits) - 1 <= len(pre_nops), "too many drain waits"
        for nop, w in zip(pre_nops, waits[:-1]):
            nop.ins.sync_info = bass_rust.SyncInfo(on_wait=[w], on_update=[])
        upd = list(si.on_update) if si.on_update else []
        drain_inst.ins.sync_info = bass_rust.SyncInfo(
            on_wait=[waits[-1]], on_update=upd)

    self.nc.all_engine_barrier()
    assert self.sems is not None
    popped = self.nc._tile_sem_poison_stack.pop()
    assert popped is self._sem_poison
    self.nc.clear_and_free_semaphores(list(self.sems.allocated().values()))
    # no trailing all_engine_barrier: NEFF completion already waits for
    # every engine's stream to end, and the sem clears land before the
    # issuing engine finishes - the barrier only stretched the measured
    # span (re-execution correctness is covered by the trace-run
    # bitwise check in test.py)


tile.TileContext._drain_and_barrier = _patched_drain_and_barrier


def _split_excess_waits(nc, max_waits=1):
    """This walrus build accepts very few sem waits per instruction (a
    TensorTensor with 2 was rejected).  Hoist all but `max_waits` of each
    instruction's waits onto same-engine NOPs inserted right before it —
    engines execute in order, so semantics are unchanged."""
    idx = 0
    for fn in nc.m.functions:
        for bb in fn.blocks:
            new = []
            changed = False
            for inst in bb.instructions:
                si = inst.sync_info
                waits = list(si.on_wait) if si is not None and si.on_wait else []
                if len(waits) > max_waits:
                    changed = True
                    for w in waits[:-max_waits]:
                        idx += 1
                        nop = mybir.InstNoOp(
                            name=f"I-waitsplit-{idx}", engine=inst.engine)
                        nop.sync_info = bass_rust.SyncInfo(
                            on_wait=[w], on_update=[])
                        new.append(nop)
                    upd = list(si.on_update) if si.on_update else []
                    inst.sync_info = bass_rust.SyncInfo(
                        on_wait=waits[-max_waits:], on_update=upd)
                new.append(inst)
            if changed:
                bb.instructions = new
# ---------------------------------------------------------------------------


def _hoist_startup_dmas(nc, n=4):
    """Move the first n no-wait load DMAs (the packed A+xc0 block and the
    lookahead x chunks) from the tile-context block to the function
    prologue, ahead of SP's boot-barrier participation.  SP then issues
    them within ~0.3 us of kernel start, so the data lands while the
    other engines are still in the ~5.5 us boot preamble (barriers +
    per-engine ucode loads).  Safe because the DMAs wait on nothing, the
    DMAHW completion semaphores start at zero, and SP-relative program
    order is preserved."""
    fn = nc.m.functions[0]
    bb0, bb1 = fn.blocks[0], fn.blocks[1]
    moved, keep = [], []
    for inst in bb1.instructions:
        if (len(moved) < n and isinstance(inst, mybir.InstDMACopy)
                and inst.engine == mybir.EngineType.SP
                and not (inst.sync_info and inst.sync_info.on_wait)):
            moved.append(inst)
        else:
            keep.append(inst)
    assert len(moved) == n, f"found only {len(moved)} hoistable DMAs"
    bb1.instructions = keep
    out0, inserted = [], False
    for inst in bb0.instructions:
        if (not inserted and inst.engine == mybir.EngineType.SP
                and isinstance(inst, mybir.InstDrain)):
            out0.extend(moved)
            inserted = True
        out0.append(inst)
    assert inserted, "no SP drain found in prologue block"
    bb0.instructions = out0


def build_program(trace_sim=False):
    nc = bass.Bass("TRN2")
    # first block: A (packed per-k) + the first x chunk, one contiguous
    # DMA so the startup-critical bytes move at full line rate
    first_d = nc.dram_tensor("first", [128, 4 * S + 4 * W0], BF,
                             kind="ExternalInput")
    xc_d = {
        ci: nc.dram_tensor(f"xc{ci}", [128, 4 * w], BF,
                           kind="ExternalInput")
        for ci, w in enumerate(CHUNKS) if ci >= 1
    }
    y_d = nc.dram_tensor("y", [len(YGRPS) * 128, max(YGRPS) * S], BF,
                         kind="ExternalOutput")

    with tile.TileContext(nc, trace_sim=trace_sim) as tc, ExitStack() as ctx:
        consts = ctx.enter_context(tc.tile_pool(name="consts", bufs=1))
        xpool = ctx.enter_context(tc.tile_pool(name="xpool", bufs=3))
        ypool = ctx.enter_context(tc.tile_pool(name="ypool", bufs=4))
        psum_y = ctx.enter_context(
            tc.tile_pool(name="psum_y", bufs=4, space="PSUM"))

        if WARM_MM:
            # PE prewarm: throwaway matmuls on a zeroed tile (results
            # never read) trip the HAM clock gate to 2.4 GHz while the
            # first loads are in flight.
            warm = consts.tile([128, 256], BF, tag="warm")
            nc.gpsimd.memset(warm, 0.0)
            for i in range(WARM_MM):
                w_ps = psum_y.tile([128, 256], FP, tag="warm_ps")
                nc.tensor.matmul(w_ps, lhsT=warm[:, :128], rhs=warm,
                                 start=True, stop=True)

        # All DMA goes through ONE HWDGE FIFO ring (two rings share the
        # 16 SDMA engines round-robin, so splitting load/store across
        # rings halves the load bandwidth mid-kernel - measured slower).
        # Emission order IS the ring schedule: the packed A+xc0 block
        # first (gates the first matmul), then a few chunks of
        # lookahead; later chunk loads are emitted from inside the loop
        # so y stores interleave with them on the ring instead of
        # queueing behind all loads.  One DMA for the whole first block:
        # splitting it [a0 a1 xc0 | a2 a3] measured ~1 us slower at
        # matched throttle - the in-order PE stalls at k=2 waiting for
        # part 2, wasting more than the early k=0,1 start saves.
        first_t = consts.tile([128, 4 * S + 4 * W0], BF, tag="first")
        nc.sync.dma_start(out=first_t, in_=first_d[:, :])
        a_off = [0, S, 2 * S, 3 * S]
        xcs = {}

        def load_chunk(ci):
            w = CHUNKS[ci]
            t = xpool.tile([128, 4 * w], BF, tag=f"xc{w}", name=f"xc{ci}")
            nc.sync.dma_start(out=t, in_=xc_d[ci][:, :])
            xcs[ci] = t

        for ci in range(1, min(LOOKAHEAD, len(CHUNKS))):
            load_chunk(ci)

        gt = 0          # global output row-tile index
        grp = 0         # output group index
        slot = 0        # row-tile slot within current group
        ybuf = None
        for ci, w in enumerate(CHUNKS):
            if ci == 0:
                xc, xoff = first_t, 4 * S
            else:
                xc, xoff = xcs[ci], 0
            for bt in range(w // 128):
                y_ps = psum_y.tile([128, S], FP, tag="y_ps")
                for k in range(4):
                    o = xoff + k * w + bt * 128
                    nc.tensor.matmul(
                        y_ps,
                        lhsT=xc[:, o:o + 128],
                        rhs=first_t[:, a_off[k]:a_off[k] + S],
                        start=(k == 0), stop=(k == 3),
                    )
                if slot == 0:
                    ybuf = ypool.tile([128, YGRPS[grp] * S], BF,
                                      tag=f"ybuf{YGRPS[grp]}")
                # alternate PSUM-drain engines so neither becomes the
                # bottleneck behind the PE
                dst = ybuf[:, slot * S:(slot + 1) * S]
                if gt % 2 == 0:
                    nc.scalar.copy(dst, y_ps)
                else:
                    nc.vector.tensor_copy(dst, y_ps)
                gt += 1
                slot += 1
                if slot == YGRPS[grp]:
                    nc.sync.dma_start(
                        out=y_d[grp * 128:(grp + 1) * 128,
                                :YGRPS[grp] * S],
                        in_=ybuf)
                    grp += 1
                    slot = 0
            if ci + LOOKAHEAD < len(CHUNKS):
                load_chunk(ci + LOOKAHEAD)
    _split_excess_waits(nc)
    # NOTE: hoisting the leading load DMAs ahead of the boot barrier
    # (_hoist_startup_dmas) was measured ineffective: the first user-DMA
    # packet never moves before ~6-8 us regardless of instruction
    # placement (the model DMA rings only start draining after the boot
    # preamble), so it is not applied.
    return nc


_NC_CACHE = {}


def _get_nc():
    if "nc" not in _NC_CACHE:
        _NC_CACHE["nc"] = build_program()
    return _NC_CACHE["nc"]


def _compute_A(vectors):
    """A = Q^T = I - V R^{-1} V^T in float64 on the host."""
    v = np.asarray(vectors, np.float64)[..., 0]        # [N, S]
    V = v.T                                            # [S, N]
    G = v @ V                                          # [N, N] = V^T V
    R = np.tril(G, -1) + np.diag((np.einsum("ns,ns->n", v, v) + EPS) / 2.0)
    Z = np.linalg.solve(R, V.T)                        # [N, S] = T V^T
    A = np.eye(S) - V @ Z                              # [S, S] = Q^T
    return A


def prepare_in_maps(x, vectors):
    # a[p, k*512 + col] = A[k*128 + p, col]
    A = _compute_A(vectors).astype(np.float32).astype(BF_NP)
    a_pk = A.reshape(4, 128, S).transpose(1, 0, 2).reshape(128, 4 * S)
    xb = np.asarray(x, dtype=np.float32).astype(BF_NP)  # [B, S] bf16
    in_maps = []
    for c in range(NCORES):
        xcore = xb[c * BPC:(c + 1) * BPC]               # [BPC, S]
        m = {}
        b0 = 0
        for ci, w in enumerate(CHUNKS):
            # xc[p, k*w + col] = x^T[k*128 + p, b0 + col]
            blk = (xcore[b0:b0 + w].reshape(w, 4, 128)  # [col, k, p]
                   .transpose(2, 1, 0).reshape(128, 4 * w))
            if ci == 0:
                m["first"] = np.ascontiguousarray(
                    np.concatenate([a_pk, blk], axis=1))
            else:
                m[f"xc{ci}"] = np.ascontiguousarray(blk)
            b0 += w
        in_maps.append(m)
    return in_maps


def _unpack_y(yarr):
    # yarr[g*128 + p, t*512 + col] -> y[rows(g) + t*128 + p, col]
    parts = []
    for g, ng in enumerate(YGRPS):
        blk = yarr[g * 128:(g + 1) * 128, :ng * S]
        parts.append(
            blk.reshape(128, ng, S).transpose(1, 0, 2).reshape(ng * 128, S))
    return np.concatenate(parts, axis=0)


def kernel(x, vectors):
    nc = _get_nc()
    in_maps = prepare_in_maps(x, vectors)
    res = run_bass_kernel_spmd(nc, in_maps, list(range(NCORES)))
    y = np.concatenate([_unpack_y(r["y"]) for r in res.results], axis=0)
    return np.ascontiguousarray(y.astype(np.float32))


if __name__ == "__main__":
    rng = np.random.default_rng(0)
    x = rng.standard_normal((B, S)).astype(np.float32)
    v = rng.standard_normal((NV, S, 1)).astype(np.float32)
    v /= np.linalg.norm(v, axis=1, keepdims=True)
    y = kernel(x, v)
    print("y", y.shape, y.dtype, float(np.abs(y).max()))
